# revision 36
# baseline (speedup 1.0000x reference)
"""Multi-head graph attention layer (GAT) for Trainium2, 8-core data-parallel.

Problem: B=8, N=1024, D_IN=256, D_OUT=64, H=8, LeakyReLU slope 0.2.
  Wh = einsum('bnd,hdf->bhnf', h, W)
  f1 = Wh @ a1, f2 = Wh @ a2              (per head)
  e  = leaky_relu(f1[:,None] + f2[None,:])
  att = softmax(where(adj==0, -inf, e))
  out = att @ Wh  -> concat heads [B, N, H*F]

Sharding: one batch element per NeuronCore (B=8 across 8 cores).

Algebra: with x = f1_i + f2_j,
  exp(leaky_relu(x)) = max(exp(x), exp(0.2 x))
                     = E1s_i * E2s_j * max(d_i * c_j, 1)
where d = exp(0.8 f1), c = exp(0.8 f2), E1s = exp(0.2 f1), E2s = exp(0.2 f2).
The E1s_i factor is constant along the softmax axis, so it cancels in the
normalization and is NEVER computed.  Per (head, j-tile) the unnormalized
attention U^T[j,i] = adj * max(d_i*c_j, 1) * E2s_j needs only TWO
DVE-class ops on the [128,1024] tile:
  1. P = (dbc * c_j) max 1     -- one 2-op tensor_scalar (4x mode), or on the
     ACT engine as R = relu(c_j*dbc - 1) (with U = (R+1)*adj split into an
     extra PE matmul against raw adj, since U = R*adj + adj).
  2. U = P * adj               -- one tensor_tensor mult (2x mode), batched
     over head PAIRS ([128, 2048] with a duplicated adjacency tile).
The E2s_j factor rides in the matmul weights [Wh*E2s | E2s]; column 64 of
the output yields the softmax denominator Z.  GPSIMD does NO elementwise
work (it shares an SBUF port with the DVE and poisons its throughput).

Finalize: ot[65,1024] PSUM -> bf16 SBUF (ACT) -> xbar DMA transpose ->
[128, 8, 65] -> batched reciprocal of Z -> eight 4x-mode tensor_scalar
muls -> single bf16 output DMA (host casts to f32).
"""

import numpy as np
import ml_dtypes

BF16 = ml_dtypes.bfloat16

B, N, D_IN, D_OUT, H = 8, 1024, 256, 64, 8
NEG_SLOPE = 0.2
P = 128                       # partitions
NJT = N // P                  # 8 j-tiles
NIT = N // P                  # 8 i-tiles
NKT = D_IN // P               # 2 k-tiles
HF = H * D_OUT                # 512
AUG = D_OUT + 1               # 65 (Wh columns + Z column)
TP = 80                       # xbar-transpose row pad (must be mult of 16)
NPAIR = H // 2                # head pairs

# ---- knobs -----------------------------------------------------------------
ACT_Y = 36        # how many of the 64 (h, jt) tiles build P on the ACT engine
XBAR_FOLD_A = False  # dma transpose row fold: False -> row r lands (p=r%128,c=r//128)


def _act_routed(h, jt):
    # spread ACT_Y tiles across (h, jt); jt 0-1 stay on the DVE so the
    # first attention tiles never queue behind ACT's phase-1 exponentials
    if jt < 2:
        return False
    return ((h + H * (jt - 2)) * 5) % 48 < ACT_Y


def _build_program():
    """Build the single-core SPMD Bass program."""
    import concourse.bass as bass
    import concourse.bacc as bacc
    import concourse.tile as tile
    from concourse import mybir
    from concourse.masks import make_identity

    f32 = mybir.dt.float32
    bf16 = mybir.dt.bfloat16
    AF = mybir.ActivationFunctionType
    OP = mybir.AluOpType

    nc = bacc.Bacc("TRN2", target_bir_lowering=False, debug=False,
                   enable_asserts=False, num_devices=8)

    hT = nc.dram_tensor("hT", [D_IN, N], bf16, kind="ExternalInput").ap()
    adjT = nc.dram_tensor("adjT", [N, N], bf16, kind="ExternalInput").ap()
    wrs = nc.dram_tensor("wrs", [D_IN, HF], bf16, kind="ExternalInput").ap()
    w12 = nc.dram_tensor("w12", [D_IN, 2 * H], bf16,
                         kind="ExternalInput").ap()
    out = nc.dram_tensor("out", [N, HF], bf16, kind="ExternalOutput").ap()

    with tile.TileContext(nc) as tc:
        with (
            tc.tile_pool(name="const", bufs=1) as const,
            tc.tile_pool(name="inputs", bufs=1) as inputs,
            tc.tile_pool(name="whp", bufs=1) as whp,
            tc.tile_pool(name="ecol", bufs=1) as ecolp,
            tc.tile_pool(name="psp", bufs=8, space="PSUM") as psp,
            tc.tile_pool(name="bcast", bufs=1) as bcastp,
            tc.tile_pool(name="u2", bufs=8) as u2p,
            tc.tile_pool(name="u2m", bufs=8) as u2mp,
            tc.tile_pool(name="work", bufs=2) as work,
            tc.tile_pool(name="fin", bufs=2) as fin,
            tc.tile_pool(name="fin2", bufs=3) as fin2,
            tc.tile_pool(name="dram", bufs=1, space="DRAM") as dramp,
        ):
            # ---- Phase 0: constants + input loads -------------------------
            ident = const.tile([P, P], f32)
            make_identity(nc, ident)
            negone = const.tile([P, 1], f32)
            nc.vector.memset(negone, -1.0)

            # small inputs first (gate the f/Wh matmuls)
            ht_sb = []
            for kt in range(NKT):
                t = inputs.tile([P, N], bf16, tag=f"ht{kt}")
                nc.sync.dma_start(out=t, in_=hT[kt * P:(kt + 1) * P, :])
                ht_sb.append(t)
            wrs_sb = []
            for kt in range(NKT):
                t = inputs.tile([P, HF], bf16, tag=f"wrs{kt}")
                nc.sync.dma_start(out=t, in_=wrs[kt * P:(kt + 1) * P, :])
                wrs_sb.append(t)
            w12_sb = []
            for kt in range(NKT):
                t = inputs.tile([P, 2 * H], bf16, tag=f"w12{kt}")
                nc.sync.dma_start(out=t, in_=w12[kt * P:(kt + 1) * P, :])
                w12_sb.append(t)
            # adjacency: single-width tiles; the mask TT reads them twice via
            # a 0-stride pair view, so no duplication traffic
            adj_sb = []
            for jt in range(NJT):
                t = inputs.tile([P, N], bf16, tag=f"adj{jt}")
                nc.sync.dma_start(out=t, in_=adjT[jt * P:(jt + 1) * P, :])
                adj_sb.append(t)

            # ---- Phase 1+2, dependency-ordered waves ----------------------
            # ecols[it][:, h]     = E2s = exp(0.2 f2_h)   (whaug scale + Z col)
            # ecols[it][:, 8 + h] = c   = exp(0.8 f2_h)   (P-build scale)
            # drow[h, i]          = d   = exp(0.8 f1_h[i])
            # whaug[it]           = [Wh * E2s | E2s]
            drow = const.tile([H, N], bf16)

            # wave A: all f matmuls, evacuated to SBUF immediately
            fsbs = []
            for it in range(NIT):
                ps = psp.tile([P, 2 * H], f32, tag='ps')
                for kt in range(NKT):
                    lhsT = ht_sb[kt][:, it * P:(it + 1) * P]
                    nc.tensor.matmul(ps, lhsT, w12_sb[kt],
                                     start=(kt == 0), stop=(kt == NKT - 1))
                fsb = ecolp.tile([P, 2 * H], f32, tag=f"fsb{it}")
                nc.vector.tensor_copy(fsb, ps)
                fsbs.append(fsb)

            # wave B: transpose f1, exponentiate into drow (gates phase 3)
            drow_dram = dramp.tile([H, N], bf16)
            for it in range(NIT):
                tr = psp.tile([2 * H, P], f32, tag='ps')
                nc.tensor.transpose(tr, fsbs[it], ident)
                nc.scalar.activation(drow[0:H, it * P:(it + 1) * P],
                                     tr[0:H, :], AF.Exp,
                                     scale=1.0 - NEG_SLOPE)
            nc.sync.dma_start(out=drow_dram, in_=drow)
            drow_flat = drow_dram.rearrange('a b -> (a b)').unsqueeze(0)

            # wave C: per-j-tile exponentials (from SBUF copies)
            ecols = []
            for it in range(NIT):
                ec = ecolp.tile([P, 2 * H], f32, tag=f"ecols{it}")
                nc.scalar.activation(ec[:, 0:H], fsbs[it][:, H:2 * H],
                                     AF.Exp, scale=NEG_SLOPE)
                nc.scalar.activation(ec[:, H:2 * H], fsbs[it][:, H:2 * H],
                                     AF.Exp, scale=1.0 - NEG_SLOPE)
                ecols.append(ec)

            # wave D: Wh matmuls + whaug (PE/DVE only, overlaps B/C)
            whaug = []
            for it in range(NIT):
                wh = psp.tile([P, HF], f32, tag='ps')
                for kt in range(NKT):
                    lhsT = ht_sb[kt][:, it * P:(it + 1) * P]
                    nc.tensor.matmul(wh, lhsT, wrs_sb[kt],
                                     start=(kt == 0), stop=(kt == NKT - 1))
                wa = whp.tile([P, H, AUG], bf16, tag=f"whaug{it}")
                nc.vector.tensor_tensor(
                    out=wa[:, :, 0:D_OUT],
                    in0=wh.rearrange('p (h f) -> p h f', f=D_OUT),
                    in1=ecols[it][:, 0:H].unsqueeze(2)
                        .broadcast_to([P, H, D_OUT]),
                    op=OP.mult)
                nc.vector.tensor_copy(wa[:, :, D_OUT], ecols[it][:, 0:H])
                whaug.append(wa)

            # full output tile: [p, c, h*64+f]; row i of the output lives at
            # (p, c) per the xbar fold (see XBAR_FOLD_A)
            out_sb = whp.tile([P, NIT, HF], bf16, tag="osb")

            # standing padded evac tiles (xbar needs rows % 16 == 0); the
            # pad rows are initialized once and never read back
            ev_tiles = []
            for k in range(2):
                evt = whp.tile([TP, N], bf16, tag=f"ev{k}")
                nc.vector.memset(evt[D_OUT:TP, :], 0.0)
                ev_tiles.append(evt)

            # ---- Phase 3: attention, one head PAIR at a time --------------
            # prefetch all pair broadcasts up front
            db2s = []
            for pr in range(NPAIR):
                db2 = bcastp.tile([P, 2 * N], bf16, tag=f"db2_{pr}")
                nc.sync.dma_start(
                    out=db2,
                    in_=drow_flat[:, 2 * pr * N:(2 * pr + 2) * N]
                        .partition_broadcast(P))
                db2s.append(db2)

            for pr in range(NPAIR):
                h0 = 2 * pr
                db2 = db2s[pr]

                ots = []
                nmm = []   # matmuls per accumulation group (per nh slice)
                for hh in range(2):
                    ota = psp.tile([AUG, 512], f32, tag="ps")
                    otb = psp.tile([AUG, 512], f32, tag="ps")
                    ots.append((ota, otb))
                    nmm.append(NJT + sum(
                        1 for jt in range(NJT) if _act_routed(h0 + hh, jt)))
                cnt = [0, 0]

                for jt in range(NJT):
                    u2 = u2p.tile([P, 2 * N], bf16, tag="u2")
                    routed = []
                    for hh in range(2):
                        h = h0 + hh
                        ccol = ecols[jt][:, H + h:H + h + 1]
                        half = u2[:, hh * N:(hh + 1) * N]
                        dhalf = db2[:, hh * N:(hh + 1) * N]
                        r = _act_routed(h, jt)
                        routed.append(r)
                        if r:
                            # R = relu(c_j * d - 1)  (the +1 rides on an
                            # extra matmul against raw adj)
                            nc.scalar.activation(half, dhalf, AF.Relu,
                                                 bias=negone, scale=ccol)
                        else:
                            # P = max(c_j * d, 1)
                            nc.vector.tensor_scalar(
                                half, dhalf, ccol, 1.0, OP.mult, OP.max)
                    u2m = u2mp.tile([P, 2 * N], bf16, tag="u2m")
                    nc.vector.tensor_tensor(
                        out=u2m.rearrange('p (x n) -> p x n', x=2),
                        in0=u2.rearrange('p (x n) -> p x n', x=2),
                        in1=adj_sb[jt].unsqueeze(1).broadcast_to([P, 2, N]),
                        op=OP.mult)
                    for hh in range(2):
                        h = h0 + hh
                        lhsT = whaug[jt][:, h, :]
                        first = cnt[hh] == 0
                        cnt[hh] += 1
                        last = cnt[hh] == nmm[hh]
                        for nh in range(2):
                            nc.tensor.matmul(
                                ots[hh][nh], lhsT,
                                u2m[:, hh * N + nh * 512:
                                    hh * N + (nh + 1) * 512],
                                start=first, stop=last)
                        if routed[hh]:
                            first = False
                            cnt[hh] += 1
                            last = cnt[hh] == nmm[hh]
                            for nh in range(2):
                                nc.tensor.matmul(
                                    ots[hh][nh], lhsT,
                                    adj_sb[jt][:, nh * 512:(nh + 1) * 512],
                                    start=False, stop=last)

                    # ---- finalize each head of the pair -------------------
                for hh in range(2):
                    h = h0 + hh
                    ev = ev_tiles[hh]
                    nc.scalar.copy(ev[0:AUG, 0:512], ots[hh][0])
                    nc.scalar.copy(ev[0:AUG, 512:1024], ots[hh][1])
                    ttile = fin2.tile([P, NIT, TP], bf16, tag="tt")
                    nc.sync.dma_start_transpose(out=ttile, in_=ev)
                    rcol = fin.tile([P, NIT], f32, tag="rcol")
                    nc.vector.reciprocal(rcol, ttile[:, :, D_OUT])
                    nc.vector.tensor_tensor(
                        out=out_sb[:, :, h * D_OUT:(h + 1) * D_OUT],
                        in0=ttile[:, :, 0:D_OUT],
                        in1=rcol.unsqueeze(2).broadcast_to([P, NIT, D_OUT]),
                        op=OP.mult)

            # ---- Phase 4: store -------------------------------------------
            if XBAR_FOLD_A:
                # transposed row r landed at (p=r//8, c=r%8)
                oview = out.rearrange("(p c) f -> p c f", c=NIT)
            else:
                # row r landed at (p=r%128, c=r//128)
                oview = out.rearrange("(c p) f -> p c f", p=P)
            nc.sync.dma_start(out=oview, in_=out_sb)

    nc.compile()
    return nc


def _host_prep(h, adj, W, a):
    """Host-side input prep: transposes / casts / tiny einsums only."""
    a1, a2 = a[:, :D_OUT], a[:, D_OUT:]
    w1 = np.einsum("hdf,hf->hd", W, a1).astype(np.float32)   # [H, D_IN]
    w2 = np.einsum("hdf,hf->hd", W, a2).astype(np.float32)
    w12 = np.concatenate([w1.T, w2.T], axis=1).astype(BF16)  # [D_IN, 16]
    wrs = np.ascontiguousarray(
        W.transpose(1, 0, 2).reshape(D_IN, HF)).astype(BF16)
    in_maps = []
    for b in range(B):
        in_maps.append({
            "hT": np.ascontiguousarray(h[b].T).astype(BF16),
            "adjT": np.ascontiguousarray(adj[b].T).astype(BF16),
            "wrs": wrs,
            "w12": w12,
        })
    return in_maps


def kernel(h, adj, W, a):
    from concourse.bass_utils import run_bass_kernel_spmd

    in_maps = _host_prep(np.asarray(h), np.asarray(adj),
                         np.asarray(W), np.asarray(a))
    nc = _build_program()
    res = run_bass_kernel_spmd(nc, in_maps, core_ids=list(range(B)))
    out = np.stack([np.asarray(res.results[b]["out"]).astype(np.float32)
                    for b in range(B)])
    return out


# revision 37
# speedup vs baseline: 1.0269x; 1.0269x over previous
"""Multi-head graph attention layer (GAT) for Trainium2, 8-core data-parallel.

Problem: B=8, N=1024, D_IN=256, D_OUT=64, H=8, LeakyReLU slope 0.2.
  Wh = einsum('bnd,hdf->bhnf', h, W)
  f1 = Wh @ a1, f2 = Wh @ a2              (per head)
  e  = leaky_relu(f1[:,None] + f2[None,:])
  att = softmax(where(adj==0, -inf, e))
  out = att @ Wh  -> concat heads [B, N, H*F]

Sharding: one batch element per NeuronCore (B=8 across 8 cores).

Algebra: with x = f1_i + f2_j,
  exp(leaky_relu(x)) = max(exp(x), exp(0.2 x))
                     = E1s_i * E2s_j * max(d_i * c_j, 1)
where d = exp(0.8 f1), c = exp(0.8 f2), E1s = exp(0.2 f1), E2s = exp(0.2 f2).
The E1s_i factor is constant along the softmax axis, so it cancels in the
normalization and is NEVER computed.  Per (head, j-tile) the unnormalized
attention U^T[j,i] = adj * max(d_i*c_j, 1) * E2s_j needs only TWO
DVE-class ops on the [128,1024] tile:
  1. P = (dbc * c_j) max 1     -- one 2-op tensor_scalar (4x mode), or on the
     ACT engine as R = relu(c_j*dbc - 1) (with U = (R+1)*adj split into an
     extra PE matmul against raw adj, since U = R*adj + adj).
  2. U = P * adj               -- one tensor_tensor mult (2x mode), batched
     over head PAIRS ([128, 2048] with a duplicated adjacency tile).
The E2s_j factor rides in the matmul weights [Wh*E2s | E2s]; column 64 of
the output yields the softmax denominator Z.  GPSIMD does NO elementwise
work (it shares an SBUF port with the DVE and poisons its throughput).

Finalize: ot[65,1024] PSUM -> bf16 SBUF (ACT) -> xbar DMA transpose ->
[128, 8, 65] -> batched reciprocal of Z -> eight 4x-mode tensor_scalar
muls -> single bf16 output DMA (host casts to f32).
"""

import numpy as np
import ml_dtypes

BF16 = ml_dtypes.bfloat16

B, N, D_IN, D_OUT, H = 8, 1024, 256, 64, 8
NEG_SLOPE = 0.2
P = 128                       # partitions
NJT = N // P                  # 8 j-tiles
NIT = N // P                  # 8 i-tiles
NKT = D_IN // P               # 2 k-tiles
HF = H * D_OUT                # 512
AUG = D_OUT + 1               # 65 (Wh columns + Z column)
TP = 80                       # xbar-transpose row pad (must be mult of 16)
NPAIR = H // 2                # head pairs

# ---- knobs -----------------------------------------------------------------
ACT_Y = 30        # how many of the 64 (h, jt) tiles build P on the ACT engine
XBAR_FOLD_A = False  # dma transpose row fold: False -> row r lands (p=r%128,c=r//128)


def _act_routed(h, jt):
    # spread ACT_Y tiles across (h, jt); jt 0-1 stay on the DVE so the
    # first attention tiles never queue behind ACT's phase-1 exponentials
    if jt < 2:
        return False
    return ((h + H * (jt - 2)) * 5) % 48 < ACT_Y


def _build_program():
    """Build the single-core SPMD Bass program."""
    import concourse.bass as bass
    import concourse.bacc as bacc
    import concourse.tile as tile
    from concourse import mybir
    from concourse.masks import make_identity

    f32 = mybir.dt.float32
    bf16 = mybir.dt.bfloat16
    AF = mybir.ActivationFunctionType
    OP = mybir.AluOpType

    nc = bacc.Bacc("TRN2", target_bir_lowering=False, debug=False,
                   enable_asserts=False, num_devices=8)

    hT = nc.dram_tensor("hT", [D_IN, N], bf16, kind="ExternalInput").ap()
    adjT = nc.dram_tensor("adjT", [N, N], bf16, kind="ExternalInput").ap()
    wrs = nc.dram_tensor("wrs", [D_IN, HF], bf16, kind="ExternalInput").ap()
    w12 = nc.dram_tensor("w12", [D_IN, 2 * H], bf16,
                         kind="ExternalInput").ap()
    out = nc.dram_tensor("out", [N, HF], bf16, kind="ExternalOutput").ap()

    with tile.TileContext(nc) as tc:
        with (
            tc.tile_pool(name="const", bufs=1) as const,
            tc.tile_pool(name="inputs", bufs=1) as inputs,
            tc.tile_pool(name="whp", bufs=1) as whp,
            tc.tile_pool(name="ecol", bufs=1) as ecolp,
            tc.tile_pool(name="psp", bufs=8, space="PSUM") as psp,
            tc.tile_pool(name="bcast", bufs=1) as bcastp,
            tc.tile_pool(name="u2", bufs=6) as u2p,
            tc.tile_pool(name="u2m", bufs=6) as u2mp,
            tc.tile_pool(name="work", bufs=2) as work,
            tc.tile_pool(name="fin", bufs=2) as fin,
            tc.tile_pool(name="fin2", bufs=2) as fin2,
            tc.tile_pool(name="dram", bufs=1, space="DRAM") as dramp,
        ):
            # ---- Phase 0: constants + input loads -------------------------
            ident = const.tile([P, P], f32)
            make_identity(nc, ident)
            negone = const.tile([P, 1], f32)
            nc.vector.memset(negone, -1.0)

            # small inputs first (gate the f/Wh matmuls)
            ht_sb = []
            for kt in range(NKT):
                t = inputs.tile([P, N], bf16, tag=f"ht{kt}")
                nc.sync.dma_start(out=t, in_=hT[kt * P:(kt + 1) * P, :])
                ht_sb.append(t)
            wrs_sb = []
            for kt in range(NKT):
                t = inputs.tile([P, HF], bf16, tag=f"wrs{kt}")
                nc.sync.dma_start(out=t, in_=wrs[kt * P:(kt + 1) * P, :])
                wrs_sb.append(t)
            w12_sb = []
            for kt in range(NKT):
                t = inputs.tile([P, 2 * H], bf16, tag=f"w12{kt}")
                nc.sync.dma_start(out=t, in_=w12[kt * P:(kt + 1) * P, :])
                w12_sb.append(t)
            # adjacency: single-width tiles; the mask TT reads them twice via
            # a 0-stride pair view, so no duplication traffic
            adj_sb = []
            for jt in range(NJT):
                t = inputs.tile([P, N], bf16, tag=f"adj{jt}")
                nc.sync.dma_start(out=t, in_=adjT[jt * P:(jt + 1) * P, :])
                adj_sb.append(t)

            # ---- Phase 1+2, dependency-ordered waves ----------------------
            # ecols[it][:, h]     = E2s = exp(0.2 f2_h)   (whaug scale + Z col)
            # ecols[it][:, 8 + h] = c   = exp(0.8 f2_h)   (P-build scale)
            # drow[h, i]          = d   = exp(0.8 f1_h[i])
            # whaug[it]           = [Wh * E2s | E2s]
            drow = const.tile([H, N], bf16)

            # wave A: all f matmuls, evacuated to SBUF immediately
            fsbs = []
            for it in range(NIT):
                ps = psp.tile([P, 2 * H], f32, tag='ps')
                for kt in range(NKT):
                    lhsT = ht_sb[kt][:, it * P:(it + 1) * P]
                    nc.tensor.matmul(ps, lhsT, w12_sb[kt],
                                     start=(kt == 0), stop=(kt == NKT - 1))
                fsb = ecolp.tile([P, 2 * H], f32, tag=f"fsb{it}")
                nc.vector.tensor_copy(fsb, ps)
                fsbs.append(fsb)

            # wave B: transpose f1, exponentiate into drow (gates phase 3)
            drow_dram = dramp.tile([H, N], bf16)
            for it in range(NIT):
                tr = psp.tile([2 * H, P], f32, tag='ps')
                nc.tensor.transpose(tr, fsbs[it], ident)
                nc.scalar.activation(drow[0:H, it * P:(it + 1) * P],
                                     tr[0:H, :], AF.Exp,
                                     scale=1.0 - NEG_SLOPE)
            nc.sync.dma_start(out=drow_dram, in_=drow)
            drow_flat = drow_dram.rearrange('a b -> (a b)').unsqueeze(0)

            # wave C: per-j-tile exponentials (from SBUF copies)
            ecols = []
            for it in range(NIT):
                ec = ecolp.tile([P, 2 * H], f32, tag=f"ecols{it}")
                nc.scalar.activation(ec[:, 0:H], fsbs[it][:, H:2 * H],
                                     AF.Exp, scale=NEG_SLOPE)
                nc.scalar.activation(ec[:, H:2 * H], fsbs[it][:, H:2 * H],
                                     AF.Exp, scale=1.0 - NEG_SLOPE)
                ecols.append(ec)

            # wave D: Wh matmuls + whaug (PE/DVE only, overlaps B/C)
            whaug = []
            for it in range(NIT):
                wh = psp.tile([P, HF], f32, tag='ps')
                for kt in range(NKT):
                    lhsT = ht_sb[kt][:, it * P:(it + 1) * P]
                    nc.tensor.matmul(wh, lhsT, wrs_sb[kt],
                                     start=(kt == 0), stop=(kt == NKT - 1))
                wa = whp.tile([P, H, AUG], bf16, tag=f"whaug{it}")
                nc.vector.tensor_tensor(
                    out=wa[:, :, 0:D_OUT],
                    in0=wh.rearrange('p (h f) -> p h f', f=D_OUT),
                    in1=ecols[it][:, 0:H].unsqueeze(2)
                        .broadcast_to([P, H, D_OUT]),
                    op=OP.mult)
                nc.vector.tensor_copy(wa[:, :, D_OUT], ecols[it][:, 0:H])
                whaug.append(wa)

            # full output tile: [p, c, h*64+f]; row i of the output lives at
            # (p, c) per the xbar fold (see XBAR_FOLD_A)
            out_sb = whp.tile([P, NIT, HF], bf16, tag="osb")

            # standing padded evac tiles (xbar needs rows % 16 == 0); the
            # pad rows are initialized once and never read back
            ev_tiles = []
            for k in range(2):
                evt = whp.tile([TP, N], bf16, tag=f"ev{k}")
                nc.vector.memset(evt[D_OUT:TP, :], 0.0)
                ev_tiles.append(evt)

            # ---- Phase 3: attention, one head PAIR at a time --------------
            # prefetch all pair broadcasts up front
            db2s = []
            for pr in range(NPAIR):
                db2 = bcastp.tile([P, 2 * N], bf16, tag=f"db2_{pr}")
                nc.sync.dma_start(
                    out=db2,
                    in_=drow_flat[:, 2 * pr * N:(2 * pr + 2) * N]
                        .partition_broadcast(P))
                db2s.append(db2)

            for pr in range(NPAIR):
                h0 = 2 * pr
                db2 = db2s[pr]

                ots = []
                nmm = []   # matmuls per accumulation group (per nh slice)
                for hh in range(2):
                    ota = psp.tile([AUG, 512], f32, tag="ps")
                    otb = psp.tile([AUG, 512], f32, tag="ps")
                    ots.append((ota, otb))
                    nmm.append(NJT + sum(
                        1 for jt in range(NJT) if _act_routed(h0 + hh, jt)))
                cnt = [0, 0]

                for jt in range(NJT):
                    u2 = u2p.tile([P, 2 * N], bf16, tag="u2")
                    routed = []
                    for hh in range(2):
                        h = h0 + hh
                        ccol = ecols[jt][:, H + h:H + h + 1]
                        half = u2[:, hh * N:(hh + 1) * N]
                        dhalf = db2[:, hh * N:(hh + 1) * N]
                        r = _act_routed(h, jt)
                        routed.append(r)
                        if r:
                            # R = relu(c_j * d - 1)  (the +1 rides on an
                            # extra matmul against raw adj)
                            nc.scalar.activation(half, dhalf, AF.Relu,
                                                 bias=negone, scale=ccol)
                        else:
                            # P = max(c_j * d, 1)
                            nc.vector.tensor_scalar(
                                half, dhalf, ccol, 1.0, OP.mult, OP.max)
                    u2m = u2mp.tile([P, 2 * N], bf16, tag="u2m")
                    nc.vector.tensor_tensor(
                        out=u2m.rearrange('p (x n) -> p x n', x=2),
                        in0=u2.rearrange('p (x n) -> p x n', x=2),
                        in1=adj_sb[jt].unsqueeze(1).broadcast_to([P, 2, N]),
                        op=OP.mult)
                    for hh in range(2):
                        h = h0 + hh
                        lhsT = whaug[jt][:, h, :]
                        first = cnt[hh] == 0
                        cnt[hh] += 1
                        last = cnt[hh] == nmm[hh]
                        for nh in range(2):
                            nc.tensor.matmul(
                                ots[hh][nh], lhsT,
                                u2m[:, hh * N + nh * 512:
                                    hh * N + (nh + 1) * 512],
                                start=first, stop=last)
                        if routed[hh]:
                            first = False
                            cnt[hh] += 1
                            last = cnt[hh] == nmm[hh]
                            for nh in range(2):
                                nc.tensor.matmul(
                                    ots[hh][nh], lhsT,
                                    adj_sb[jt][:, nh * 512:(nh + 1) * 512],
                                    start=False, stop=last)

                    # ---- finalize each head of the pair -------------------
                for hh in range(2):
                    h = h0 + hh
                    ev = ev_tiles[hh]
                    nc.scalar.copy(ev[0:AUG, 0:512], ots[hh][0])
                    nc.scalar.copy(ev[0:AUG, 512:1024], ots[hh][1])
                    ttile = fin2.tile([P, NIT, TP], bf16, tag="tt")
                    nc.sync.dma_start_transpose(out=ttile, in_=ev)
                    rcol = fin.tile([P, NIT], f32, tag="rcol")
                    nc.vector.reciprocal(rcol, ttile[:, :, D_OUT])
                    nc.vector.tensor_tensor(
                        out=out_sb[:, :, h * D_OUT:(h + 1) * D_OUT],
                        in0=ttile[:, :, 0:D_OUT],
                        in1=rcol.unsqueeze(2).broadcast_to([P, NIT, D_OUT]),
                        op=OP.mult)

            # ---- Phase 4: store -------------------------------------------
            if XBAR_FOLD_A:
                # transposed row r landed at (p=r//8, c=r%8)
                oview = out.rearrange("(p c) f -> p c f", c=NIT)
            else:
                # row r landed at (p=r%128, c=r//128)
                oview = out.rearrange("(c p) f -> p c f", p=P)
            nc.sync.dma_start(out=oview, in_=out_sb)

    nc.compile()
    return nc


def _host_prep(h, adj, W, a):
    """Host-side input prep: transposes / casts / tiny einsums only."""
    a1, a2 = a[:, :D_OUT], a[:, D_OUT:]
    w1 = np.einsum("hdf,hf->hd", W, a1).astype(np.float32)   # [H, D_IN]
    w2 = np.einsum("hdf,hf->hd", W, a2).astype(np.float32)
    w12 = np.concatenate([w1.T, w2.T], axis=1).astype(BF16)  # [D_IN, 16]
    wrs = np.ascontiguousarray(
        W.transpose(1, 0, 2).reshape(D_IN, HF)).astype(BF16)
    in_maps = []
    for b in range(B):
        in_maps.append({
            "hT": np.ascontiguousarray(h[b].T).astype(BF16),
            "adjT": np.ascontiguousarray(adj[b].T).astype(BF16),
            "wrs": wrs,
            "w12": w12,
        })
    return in_maps


def kernel(h, adj, W, a):
    from concourse.bass_utils import run_bass_kernel_spmd

    in_maps = _host_prep(np.asarray(h), np.asarray(adj),
                         np.asarray(W), np.asarray(a))
    nc = _build_program()
    res = run_bass_kernel_spmd(nc, in_maps, core_ids=list(range(B)))
    out = np.stack([np.asarray(res.results[b]["out"]).astype(np.float32)
                    for b in range(B)])
    return out


# revision 40
# speedup vs baseline: 1.0381x; 1.0109x over previous
"""Multi-head graph attention layer (GAT) for Trainium2, 8-core data-parallel.

Problem: B=8, N=1024, D_IN=256, D_OUT=64, H=8, LeakyReLU slope 0.2.
  Wh = einsum('bnd,hdf->bhnf', h, W)
  f1 = Wh @ a1, f2 = Wh @ a2              (per head)
  e  = leaky_relu(f1[:,None] + f2[None,:])
  att = softmax(where(adj==0, -inf, e))
  out = att @ Wh  -> concat heads [B, N, H*F]

Sharding: one batch element per NeuronCore (B=8 across 8 cores).

Algebra: with x = f1_i + f2_j,
  exp(leaky_relu(x)) = max(exp(x), exp(0.2 x))
                     = E1s_i * E2s_j * max(d_i * c_j, 1)
where d = exp(0.8 f1), c = exp(0.8 f2), E1s = exp(0.2 f1), E2s = exp(0.2 f2).
The E1s_i factor is constant along the softmax axis, so it cancels in the
normalization and is NEVER computed.  Per (head, j-tile) the unnormalized
attention U^T[j,i] = adj * max(d_i*c_j, 1) * E2s_j needs only TWO
DVE-class ops on the [128,1024] tile:
  1. P = (dbc * c_j) max 1     -- one 2-op tensor_scalar (4x mode), or on the
     ACT engine as R = relu(c_j*dbc - 1) (with U = (R+1)*adj split into an
     extra PE matmul against raw adj, since U = R*adj + adj).
  2. U = P * adj               -- one tensor_tensor mult (2x mode), batched
     over head PAIRS ([128, 2048] with a duplicated adjacency tile).
The E2s_j factor rides in the matmul weights [Wh*E2s | E2s]; column 64 of
the output yields the softmax denominator Z.  GPSIMD does NO elementwise
work (it shares an SBUF port with the DVE and poisons its throughput).

Finalize: ot[65,1024] PSUM -> bf16 SBUF (ACT) -> xbar DMA transpose ->
[128, 8, 65] -> batched reciprocal of Z -> eight 4x-mode tensor_scalar
muls -> single bf16 output DMA (host casts to f32).
"""

import numpy as np
import ml_dtypes

BF16 = ml_dtypes.bfloat16

B, N, D_IN, D_OUT, H = 8, 1024, 256, 64, 8
NEG_SLOPE = 0.2
P = 128                       # partitions
NJT = N // P                  # 8 j-tiles
NIT = N // P                  # 8 i-tiles
NKT = D_IN // P               # 2 k-tiles
HF = H * D_OUT                # 512
AUG = D_OUT + 1               # 65 (Wh columns + Z column)
TP = 80                       # xbar-transpose row pad (must be mult of 16)
NPAIR = H // 2                # head pairs

# ---- knobs -----------------------------------------------------------------
ACT_Y = 30        # how many of the 64 (h, jt) tiles build P on the ACT engine
XBAR_FOLD_A = False  # dma transpose row fold: False -> row r lands (p=r%128,c=r//128)


def _act_routed(h, jt):
    # spread ACT_Y tiles across (h, jt); jt 0-1 stay on the DVE so the
    # first attention tiles never queue behind ACT's phase-1 exponentials
    if jt < 2:
        return False
    return ((h + H * (jt - 2)) * 5) % 48 < ACT_Y


def _build_program():
    """Build the single-core SPMD Bass program."""
    import concourse.bass as bass
    import concourse.bacc as bacc
    import concourse.tile as tile
    from concourse import mybir
    from concourse.masks import make_identity

    f32 = mybir.dt.float32
    bf16 = mybir.dt.bfloat16
    AF = mybir.ActivationFunctionType
    OP = mybir.AluOpType

    nc = bacc.Bacc("TRN2", target_bir_lowering=False, debug=False,
                   enable_asserts=False, num_devices=8)

    hT = nc.dram_tensor("hT", [D_IN, N], bf16, kind="ExternalInput").ap()
    adjT = nc.dram_tensor("adjT", [N, N], bf16, kind="ExternalInput").ap()
    wrs = nc.dram_tensor("wrs", [D_IN, HF], bf16, kind="ExternalInput").ap()
    w12 = nc.dram_tensor("w12", [D_IN, 2 * H], bf16,
                         kind="ExternalInput").ap()
    out = nc.dram_tensor("out", [N, HF], bf16, kind="ExternalOutput").ap()

    with tile.TileContext(nc) as tc:
        with (
            tc.tile_pool(name="const", bufs=1) as const,
            tc.tile_pool(name="inputs", bufs=1) as inputs,
            tc.tile_pool(name="whp", bufs=1) as whp,
            tc.tile_pool(name="ecol", bufs=1) as ecolp,
            tc.tile_pool(name="psp", bufs=8, space="PSUM") as psp,
            tc.tile_pool(name="bcast", bufs=1) as bcastp,
            tc.tile_pool(name="u2", bufs=6) as u2p,
            tc.tile_pool(name="u2m", bufs=6) as u2mp,
            tc.tile_pool(name="work", bufs=2) as work,
            tc.tile_pool(name="fin", bufs=2) as fin,
            tc.tile_pool(name="fin2", bufs=2) as fin2,
            tc.tile_pool(name="dram", bufs=1, space="DRAM") as dramp,
        ):
            # ---- Phase 0: constants + input loads -------------------------
            ident = const.tile([P, P], f32)
            make_identity(nc, ident)
            negone = const.tile([P, 1], f32)
            nc.vector.memset(negone, -1.0)

            # small inputs first (gate the f/Wh matmuls)
            ht_sb = []
            for kt in range(NKT):
                t = inputs.tile([P, N], bf16, tag=f"ht{kt}")
                nc.sync.dma_start(out=t, in_=hT[kt * P:(kt + 1) * P, :])
                ht_sb.append(t)
            wrs_sb = []
            for kt in range(NKT):
                t = inputs.tile([P, HF], bf16, tag=f"wrs{kt}")
                nc.sync.dma_start(out=t, in_=wrs[kt * P:(kt + 1) * P, :])
                wrs_sb.append(t)
            w12_sb = []
            for kt in range(NKT):
                t = inputs.tile([P, 2 * H], bf16, tag=f"w12{kt}")
                nc.sync.dma_start(out=t, in_=w12[kt * P:(kt + 1) * P, :])
                w12_sb.append(t)
            # adjacency: single-width tiles; the mask TT reads them twice via
            # a 0-stride pair view, so no duplication traffic
            adj_sb = []
            for jt in range(NJT):
                t = inputs.tile([P, N], bf16, tag=f"adj{jt}")
                nc.sync.dma_start(out=t, in_=adjT[jt * P:(jt + 1) * P, :])
                adj_sb.append(t)

            # ---- Phase 1+2, dependency-ordered waves ----------------------
            # ecols[it][:, h]     = E2s = exp(0.2 f2_h)   (whaug scale + Z col)
            # ecols[it][:, 8 + h] = c   = exp(0.8 f2_h)   (P-build scale)
            # drow[h, i]          = d   = exp(0.8 f1_h[i])
            # whaug[it]           = [Wh * E2s | E2s]
            drow = const.tile([H, N], bf16)

            # wave A: all f matmuls, evacuated to SBUF immediately
            fsbs = []
            for it in range(NIT):
                ps = psp.tile([P, 2 * H], f32, tag='ps')
                for kt in range(NKT):
                    lhsT = ht_sb[kt][:, it * P:(it + 1) * P]
                    nc.tensor.matmul(ps, lhsT, w12_sb[kt],
                                     start=(kt == 0), stop=(kt == NKT - 1))
                fsb = ecolp.tile([P, 2 * H], f32, tag=f"fsb{it}")
                nc.vector.tensor_copy(fsb, ps)
                fsbs.append(fsb)

            # wave B: transpose f1, exponentiate into drow (gates phase 3)
            drow_dram = dramp.tile([H, N], bf16)
            for it in range(NIT):
                tr = psp.tile([2 * H, P], f32, tag='ps')
                nc.tensor.transpose(tr, fsbs[it], ident)
                nc.scalar.activation(drow[0:H, it * P:(it + 1) * P],
                                     tr[0:H, :], AF.Exp,
                                     scale=1.0 - NEG_SLOPE)
            nc.sync.dma_start(out=drow_dram, in_=drow)
            drow_flat = drow_dram.rearrange('a b -> (a b)').unsqueeze(0)

            # wave C: per-j-tile exponentials (from SBUF copies)
            ecols = []
            for it in range(NIT):
                ec = ecolp.tile([P, 2 * H], f32, tag=f"ecols{it}")
                nc.scalar.activation(ec[:, 0:H], fsbs[it][:, H:2 * H],
                                     AF.Exp, scale=NEG_SLOPE)
                nc.scalar.activation(ec[:, H:2 * H], fsbs[it][:, H:2 * H],
                                     AF.Exp, scale=1.0 - NEG_SLOPE)
                ecols.append(ec)

            # wave D: Wh matmuls + whaug (PE/DVE only, overlaps B/C)
            whaug = []
            for it in range(NIT):
                wh = psp.tile([P, HF], f32, tag='ps')
                for kt in range(NKT):
                    lhsT = ht_sb[kt][:, it * P:(it + 1) * P]
                    nc.tensor.matmul(wh, lhsT, wrs_sb[kt],
                                     start=(kt == 0), stop=(kt == NKT - 1))
                wa = whp.tile([P, H, AUG], bf16, tag=f"whaug{it}")
                nc.vector.tensor_tensor(
                    out=wa[:, :, 0:D_OUT],
                    in0=wh.rearrange('p (h f) -> p h f', f=D_OUT),
                    in1=ecols[it][:, 0:H].unsqueeze(2)
                        .broadcast_to([P, H, D_OUT]),
                    op=OP.mult)
                nc.vector.tensor_copy(wa[:, :, D_OUT], ecols[it][:, 0:H])
                whaug.append(wa)

            # full output tile: [p, c, h*64+f]; row i of the output lives at
            # (p, c) per the xbar fold (see XBAR_FOLD_A)
            out_sb = whp.tile([P, NIT, HF], bf16, tag="osb")

            # standing padded evac tiles (xbar needs rows % 16 == 0); the
            # pad rows are initialized once and never read back
            ev_tiles = []
            for k in range(2):
                evt = whp.tile([TP, N], bf16, tag=f"ev{k}")
                nc.vector.memset(evt[D_OUT:TP, :], 0.0)
                ev_tiles.append(evt)

            # ---- Phase 3: attention, one head PAIR at a time --------------
            if XBAR_FOLD_A:
                # transposed row r landed at (p=r//8, c=r%8)
                oview = out.rearrange("(p c) f -> p c f", c=NIT)
            else:
                # row r landed at (p=r%128, c=r//128)
                oview = out.rearrange("(c p) f -> p c f", p=P)

            # prefetch all pair broadcasts up front
            db2s = []
            for pr in range(NPAIR):
                db2 = bcastp.tile([P, 2 * N], bf16, tag=f"db2_{pr}")
                nc.sync.dma_start(
                    out=db2,
                    in_=drow_flat[:, 2 * pr * N:(2 * pr + 2) * N]
                        .partition_broadcast(P))
                db2s.append(db2)

            for pr in range(NPAIR):
                h0 = 2 * pr
                db2 = db2s[pr]

                ots = []
                has_adj = []
                for hh in range(2):
                    ota = psp.tile([AUG, 512], f32, tag="ps")
                    otb = psp.tile([AUG, 512], f32, tag="ps")
                    ots.append((ota, otb))
                    has_adj.append(any(
                        _act_routed(h0 + hh, jt) for jt in range(NJT)))

                # the adjacency-term matmuls of ACT-routed tiles depend only
                # on whaug and adj, not on U — run them all up front so the
                # PE has work while the first U tiles are being built
                for jt in range(NJT):
                    for hh in range(2):
                        h = h0 + hh
                        if not _act_routed(h, jt):
                            continue
                        lhsT = whaug[jt][:, h, :]
                        first = not any(_act_routed(h, j) for j in range(jt))
                        for nh in range(2):
                            nc.tensor.matmul(
                                ots[hh][nh], lhsT,
                                adj_sb[jt][:, nh * 512:(nh + 1) * 512],
                                start=first, stop=False)

                for jt in range(NJT):
                    u2 = u2p.tile([P, 2 * N], bf16, tag="u2")
                    for hh in range(2):
                        h = h0 + hh
                        ccol = ecols[jt][:, H + h:H + h + 1]
                        half = u2[:, hh * N:(hh + 1) * N]
                        dhalf = db2[:, hh * N:(hh + 1) * N]
                        if _act_routed(h, jt):
                            # R = relu(c_j * d - 1)  (the +1 rode on the
                            # up-front matmul against raw adj)
                            nc.scalar.activation(half, dhalf, AF.Relu,
                                                 bias=negone, scale=ccol)
                        else:
                            # P = max(c_j * d, 1)
                            nc.vector.tensor_scalar(
                                half, dhalf, ccol, 1.0, OP.mult, OP.max)
                    u2m = u2mp.tile([P, 2 * N], bf16, tag="u2m")
                    nc.vector.tensor_tensor(
                        out=u2m.rearrange('p (x n) -> p x n', x=2),
                        in0=u2.rearrange('p (x n) -> p x n', x=2),
                        in1=adj_sb[jt].unsqueeze(1).broadcast_to([P, 2, N]),
                        op=OP.mult)
                    for hh in range(2):
                        h = h0 + hh
                        lhsT = whaug[jt][:, h, :]
                        for nh in range(2):
                            nc.tensor.matmul(
                                ots[hh][nh], lhsT,
                                u2m[:, hh * N + nh * 512:
                                    hh * N + (nh + 1) * 512],
                                start=(jt == 0 and not has_adj[hh]),
                                stop=(jt == NJT - 1))

                    # ---- finalize each head of the pair -------------------
                for hh in range(2):
                    h = h0 + hh
                    ev = ev_tiles[hh]
                    nc.scalar.copy(ev[0:AUG, 0:512], ots[hh][0])
                    nc.scalar.copy(ev[0:AUG, 512:1024], ots[hh][1])
                    ttile = fin2.tile([P, NIT, TP], bf16, tag="tt")
                    nc.sync.dma_start_transpose(out=ttile, in_=ev)
                    rcol = fin.tile([P, NIT], f32, tag="rcol")
                    nc.vector.reciprocal(rcol, ttile[:, :, D_OUT])
                    nc.vector.tensor_tensor(
                        out=out_sb[:, :, h * D_OUT:(h + 1) * D_OUT],
                        in0=ttile[:, :, 0:D_OUT],
                        in1=rcol.unsqueeze(2).broadcast_to([P, NIT, D_OUT]),
                        op=OP.mult)

                if pr == NPAIR - 2:
                    # heads 0-5 are final: stream most of the output out
                    # while the last pair is still computing
                    nc.sync.dma_start(
                        out=oview[:, :, 0:6 * D_OUT],
                        in_=out_sb[:, :, 0:6 * D_OUT])

            # ---- Phase 4: store the last pair's columns -------------------
            nc.sync.dma_start(out=oview[:, :, 6 * D_OUT:HF],
                              in_=out_sb[:, :, 6 * D_OUT:HF])

    nc.compile()
    return nc


def _host_prep(h, adj, W, a):
    """Host-side input prep: transposes / casts / tiny einsums only."""
    a1, a2 = a[:, :D_OUT], a[:, D_OUT:]
    w1 = np.einsum("hdf,hf->hd", W, a1).astype(np.float32)   # [H, D_IN]
    w2 = np.einsum("hdf,hf->hd", W, a2).astype(np.float32)
    w12 = np.concatenate([w1.T, w2.T], axis=1).astype(BF16)  # [D_IN, 16]
    wrs = np.ascontiguousarray(
        W.transpose(1, 0, 2).reshape(D_IN, HF)).astype(BF16)
    in_maps = []
    for b in range(B):
        in_maps.append({
            "hT": np.ascontiguousarray(h[b].T).astype(BF16),
            "adjT": np.ascontiguousarray(adj[b].T).astype(BF16),
            "wrs": wrs,
            "w12": w12,
        })
    return in_maps


def kernel(h, adj, W, a):
    from concourse.bass_utils import run_bass_kernel_spmd

    in_maps = _host_prep(np.asarray(h), np.asarray(adj),
                         np.asarray(W), np.asarray(a))
    nc = _build_program()
    res = run_bass_kernel_spmd(nc, in_maps, core_ids=list(range(B)))
    out = np.stack([np.asarray(res.results[b]["out"]).astype(np.float32)
                    for b in range(B)])
    return out


# revision 45
# speedup vs baseline: 1.0587x; 1.0198x over previous
"""Multi-head graph attention layer (GAT) for Trainium2, 8-core data-parallel.

Problem: B=8, N=1024, D_IN=256, D_OUT=64, H=8, LeakyReLU slope 0.2.
  Wh = einsum('bnd,hdf->bhnf', h, W)
  f1 = Wh @ a1, f2 = Wh @ a2              (per head)
  e  = leaky_relu(f1[:,None] + f2[None,:])
  att = softmax(where(adj==0, -inf, e))
  out = att @ Wh  -> concat heads [B, N, H*F]

Sharding: one batch element per NeuronCore (B=8 across 8 cores).

Algebra: with x = f1_i + f2_j,
  exp(leaky_relu(x)) = max(exp(x), exp(0.2 x))
                     = E1s_i * E2s_j * max(d_i * c_j, 1)
where d = exp(0.8 f1), c = exp(0.8 f2), E1s = exp(0.2 f1), E2s = exp(0.2 f2).
The E1s_i factor is constant along the softmax axis, so it cancels in the
normalization and is NEVER computed.  Per (head, j-tile) the unnormalized
attention U^T[j,i] = adj * max(d_i*c_j, 1) * E2s_j needs only TWO
DVE-class ops on the [128,1024] tile:
  1. P = (dbc * c_j) max 1     -- one 2-op tensor_scalar (4x mode), or on the
     ACT engine as R = relu(c_j*dbc - 1) (with U = (R+1)*adj split into an
     extra PE matmul against raw adj, since U = R*adj + adj).
  2. U = P * adj               -- one tensor_tensor mult (2x mode), batched
     over head PAIRS ([128, 2048] with a duplicated adjacency tile).
The E2s_j factor rides in the matmul weights [Wh*E2s | E2s]; column 64 of
the output yields the softmax denominator Z.  GPSIMD does NO elementwise
work (it shares an SBUF port with the DVE and poisons its throughput).

Finalize: ot[65,1024] PSUM -> bf16 SBUF (ACT) -> xbar DMA transpose ->
[128, 8, 65] -> batched reciprocal of Z -> eight 4x-mode tensor_scalar
muls -> single bf16 output DMA (host casts to f32).
"""

import numpy as np
import ml_dtypes

BF16 = ml_dtypes.bfloat16

B, N, D_IN, D_OUT, H = 8, 1024, 256, 64, 8
NEG_SLOPE = 0.2
P = 128                       # partitions
NJT = N // P                  # 8 j-tiles
NIT = N // P                  # 8 i-tiles
NKT = D_IN // P               # 2 k-tiles
HF = H * D_OUT                # 512
AUG = D_OUT + 1               # 65 (Wh columns + Z column)
TP = 80                       # xbar-transpose row pad (must be mult of 16)
NPAIR = H // 2                # head pairs

# ---- knobs -----------------------------------------------------------------
ACT_Y = 30        # how many of the 64 (h, jt) tiles build P on the ACT engine
XBAR_FOLD_A = False  # dma transpose row fold: False -> row r lands (p=r%128,c=r//128)


def _act_routed(h, jt):
    # spread ACT_Y tiles across (h, jt); jt 0-1 stay on the DVE so the
    # first attention tiles never queue behind ACT's phase-1 exponentials
    if jt < 2:
        return False
    return ((h + H * (jt - 2)) * 5) % 48 < ACT_Y


def _build_program():
    """Build the single-core SPMD Bass program."""
    import concourse.bass as bass
    import concourse.bacc as bacc
    import concourse.tile as tile
    from concourse import mybir
    from concourse.masks import make_identity

    f32 = mybir.dt.float32
    bf16 = mybir.dt.bfloat16
    AF = mybir.ActivationFunctionType
    OP = mybir.AluOpType

    nc = bacc.Bacc("TRN2", target_bir_lowering=False, debug=False,
                   enable_asserts=False, num_devices=8)

    hT = nc.dram_tensor("hT", [D_IN, N], bf16, kind="ExternalInput").ap()
    adjT = nc.dram_tensor("adjT", [N, N], bf16, kind="ExternalInput").ap()
    wrs = nc.dram_tensor("wrs", [D_IN, HF], bf16, kind="ExternalInput").ap()
    w12 = nc.dram_tensor("w12", [D_IN, 2 * H], bf16,
                         kind="ExternalInput").ap()
    # per-pair contiguous blocks: out[pr*P+p, c*2*D_OUT + hh*D_OUT + f],
    # unscrambled on the host (kernel() below)
    out = nc.dram_tensor("out", [NPAIR * P, NIT * 2 * D_OUT], bf16,
                         kind="ExternalOutput").ap()

    with tile.TileContext(nc) as tc:
        with (
            tc.tile_pool(name="const", bufs=1) as const,
            tc.tile_pool(name="inputs", bufs=1) as inputs,
            tc.tile_pool(name="whp", bufs=1) as whp,
            tc.tile_pool(name="ecol", bufs=1) as ecolp,
            tc.tile_pool(name="psp", bufs=8, space="PSUM") as psp,
            tc.tile_pool(name="bcast", bufs=1) as bcastp,
            tc.tile_pool(name="u2", bufs=6) as u2p,
            tc.tile_pool(name="u2m", bufs=6) as u2mp,
            tc.tile_pool(name="work", bufs=2) as work,
            tc.tile_pool(name="fin", bufs=2) as fin,
            tc.tile_pool(name="fin2", bufs=2) as fin2,
            tc.tile_pool(name="dram", bufs=1, space="DRAM") as dramp,
        ):
            # ---- Phase 0: constants + input loads -------------------------
            ident = const.tile([P, P], f32)
            make_identity(nc, ident)
            negone = const.tile([P, 1], f32)
            nc.vector.memset(negone, -1.0)

            # small inputs first (gate the f/Wh matmuls)
            ht_sb = []
            for kt in range(NKT):
                t = inputs.tile([P, N], bf16, tag=f"ht{kt}")
                nc.sync.dma_start(out=t, in_=hT[kt * P:(kt + 1) * P, :])
                ht_sb.append(t)
            wrs_sb = []
            for kt in range(NKT):
                t = inputs.tile([P, HF], bf16, tag=f"wrs{kt}")
                nc.sync.dma_start(out=t, in_=wrs[kt * P:(kt + 1) * P, :])
                wrs_sb.append(t)
            w12_sb = []
            for kt in range(NKT):
                t = inputs.tile([P, 2 * H], bf16, tag=f"w12{kt}")
                nc.sync.dma_start(out=t, in_=w12[kt * P:(kt + 1) * P, :])
                w12_sb.append(t)
            # adjacency: single-width tiles; the mask TT reads them twice via
            # a 0-stride pair view, so no duplication traffic
            adj_sb = []
            for jt in range(NJT):
                t = inputs.tile([P, N], bf16, tag=f"adj{jt}")
                nc.sync.dma_start(out=t, in_=adjT[jt * P:(jt + 1) * P, :])
                adj_sb.append(t)

            # ---- Phase 1+2, dependency-ordered waves ----------------------
            # ecols[it][:, h]     = E2s = exp(0.2 f2_h)   (whaug scale + Z col)
            # ecols[it][:, 8 + h] = c   = exp(0.8 f2_h)   (P-build scale)
            # drow[h, i]          = d   = exp(0.8 f1_h[i])
            # whaug[it]           = [Wh * E2s | E2s]
            drow = const.tile([H, N], bf16)

            # wave A: all f matmuls, evacuated to SBUF immediately
            fsbs = []
            for it in range(NIT):
                ps = psp.tile([P, 2 * H], f32, tag='ps')
                for kt in range(NKT):
                    lhsT = ht_sb[kt][:, it * P:(it + 1) * P]
                    nc.tensor.matmul(ps, lhsT, w12_sb[kt],
                                     start=(kt == 0), stop=(kt == NKT - 1))
                fsb = ecolp.tile([P, 2 * H], f32, tag=f"fsb{it}")
                nc.vector.tensor_copy(fsb, ps)
                fsbs.append(fsb)

            # wave B: transpose f1, exponentiate into drow (gates phase 3)
            drow_dram = dramp.tile([H, N], bf16)
            for it in range(NIT):
                tr = psp.tile([2 * H, P], f32, tag='ps')
                nc.tensor.transpose(tr, fsbs[it], ident)
                nc.scalar.activation(drow[0:H, it * P:(it + 1) * P],
                                     tr[0:H, :], AF.Exp,
                                     scale=1.0 - NEG_SLOPE)
            nc.sync.dma_start(out=drow_dram, in_=drow)
            drow_flat = drow_dram.rearrange('a b -> (a b)').unsqueeze(0)

            # wave C: per-j-tile exponentials (from SBUF copies)
            ecols = []
            for it in range(NIT):
                ec = ecolp.tile([P, 2 * H], f32, tag=f"ecols{it}")
                nc.scalar.activation(ec[:, 0:H], fsbs[it][:, H:2 * H],
                                     AF.Exp, scale=NEG_SLOPE)
                nc.scalar.activation(ec[:, H:2 * H], fsbs[it][:, H:2 * H],
                                     AF.Exp, scale=1.0 - NEG_SLOPE)
                ecols.append(ec)

            # wave D: Wh matmuls + whaug (PE/DVE only, overlaps B/C)
            whaug = []
            for it in range(NIT):
                wh = psp.tile([P, HF], f32, tag='ps')
                for kt in range(NKT):
                    lhsT = ht_sb[kt][:, it * P:(it + 1) * P]
                    nc.tensor.matmul(wh, lhsT, wrs_sb[kt],
                                     start=(kt == 0), stop=(kt == NKT - 1))
                wa = whp.tile([P, H, AUG], bf16, tag=f"whaug{it}")
                nc.vector.tensor_tensor(
                    out=wa[:, :, 0:D_OUT],
                    in0=wh.rearrange('p (h f) -> p h f', f=D_OUT),
                    in1=ecols[it][:, 0:H].unsqueeze(2)
                        .broadcast_to([P, H, D_OUT]),
                    op=OP.mult)
                nc.vector.tensor_copy(wa[:, :, D_OUT], ecols[it][:, 0:H])
                whaug.append(wa)

            # per-pair output tiles: [p, c, hh*64+f]; row i of the output
            # lives at (p, c) per the xbar fold (see XBAR_FOLD_A)
            out_sbs = []
            for pr in range(NPAIR):
                osb = whp.tile([P, NIT, 2 * D_OUT], bf16, tag=f"osb{pr}")
                out_sbs.append(osb)

            # standing padded evac tiles (xbar needs rows % 16 == 0); the
            # pad rows are initialized once and never read back
            ev_tiles = []
            for k in range(2):
                evt = whp.tile([TP, N], bf16, tag=f"ev{k}")
                nc.vector.memset(evt[D_OUT:TP, :], 0.0)
                ev_tiles.append(evt)

            # ---- Phase 3: attention, one head PAIR at a time --------------
            # prefetch all pair broadcasts up front
            db2s = []
            for pr in range(NPAIR):
                db2 = bcastp.tile([P, 2 * N], bf16, tag=f"db2_{pr}")
                nc.sync.dma_start(
                    out=db2,
                    in_=drow_flat[:, 2 * pr * N:(2 * pr + 2) * N]
                        .partition_broadcast(P))
                db2s.append(db2)

            for pr in range(NPAIR):
                h0 = 2 * pr
                db2 = db2s[pr]

                ots = []
                has_adj = []
                for hh in range(2):
                    ota = psp.tile([AUG, 512], f32, tag="ps")
                    otb = psp.tile([AUG, 512], f32, tag="ps")
                    ots.append((ota, otb))
                    has_adj.append(any(
                        _act_routed(h0 + hh, jt) for jt in range(NJT)))

                # the adjacency-term matmuls of ACT-routed tiles depend only
                # on whaug and adj, not on U — run them all up front so the
                # PE has work while the first U tiles are being built
                for jt in range(NJT):
                    for hh in range(2):
                        h = h0 + hh
                        if not _act_routed(h, jt):
                            continue
                        lhsT = whaug[jt][:, h, :]
                        first = not any(_act_routed(h, j) for j in range(jt))
                        for nh in range(2):
                            nc.tensor.matmul(
                                ots[hh][nh], lhsT,
                                adj_sb[jt][:, nh * 512:(nh + 1) * 512],
                                start=first, stop=False)

                for jt in range(NJT):
                    u2 = u2p.tile([P, 2 * N], bf16, tag="u2")
                    for hh in range(2):
                        h = h0 + hh
                        ccol = ecols[jt][:, H + h:H + h + 1]
                        half = u2[:, hh * N:(hh + 1) * N]
                        dhalf = db2[:, hh * N:(hh + 1) * N]
                        if _act_routed(h, jt):
                            # R = relu(c_j * d - 1)  (the +1 rode on the
                            # up-front matmul against raw adj)
                            nc.scalar.activation(half, dhalf, AF.Relu,
                                                 bias=negone, scale=ccol)
                        else:
                            # P = max(c_j * d, 1)
                            nc.vector.tensor_scalar(
                                half, dhalf, ccol, 1.0, OP.mult, OP.max)
                    u2m = u2mp.tile([P, 2 * N], bf16, tag="u2m")
                    nc.vector.tensor_tensor(
                        out=u2m.rearrange('p (x n) -> p x n', x=2),
                        in0=u2.rearrange('p (x n) -> p x n', x=2),
                        in1=adj_sb[jt].unsqueeze(1).broadcast_to([P, 2, N]),
                        op=OP.mult)
                    for hh in range(2):
                        h = h0 + hh
                        lhsT = whaug[jt][:, h, :]
                        for nh in range(2):
                            nc.tensor.matmul(
                                ots[hh][nh], lhsT,
                                u2m[:, hh * N + nh * 512:
                                    hh * N + (nh + 1) * 512],
                                start=(jt == 0 and not has_adj[hh]),
                                stop=(jt == NJT - 1))

                    # ---- finalize each head of the pair -------------------
                for hh in range(2):
                    h = h0 + hh
                    ev = ev_tiles[hh]
                    nc.scalar.copy(ev[0:AUG, 0:512], ots[hh][0])
                    nc.scalar.copy(ev[0:AUG, 512:1024], ots[hh][1])
                    ttile = fin2.tile([P, NIT, TP], bf16, tag="tt")
                    nc.sync.dma_start_transpose(out=ttile, in_=ev)
                    rcol = fin.tile([P, NIT], f32, tag="rcol")
                    nc.vector.reciprocal(rcol, ttile[:, :, D_OUT])
                    nc.vector.tensor_tensor(
                        out=out_sbs[pr][:, :, hh * D_OUT:(hh + 1) * D_OUT],
                        in0=ttile[:, :, 0:D_OUT],
                        in1=rcol.unsqueeze(2).broadcast_to([P, NIT, D_OUT]),
                        op=OP.mult)

                # this pair's block is final: stream it out (fully
                # contiguous on both sides -> 2KB descriptor rows)
                nc.sync.dma_start(out=out[pr * P:(pr + 1) * P, :],
                                  in_=out_sbs[pr])

    nc.compile()
    return nc


def _host_prep(h, adj, W, a):
    """Host-side input prep: transposes / casts / tiny einsums only."""
    a1, a2 = a[:, :D_OUT], a[:, D_OUT:]
    w1 = np.einsum("hdf,hf->hd", W, a1).astype(np.float32)   # [H, D_IN]
    w2 = np.einsum("hdf,hf->hd", W, a2).astype(np.float32)
    w12 = np.concatenate([w1.T, w2.T], axis=1).astype(BF16)  # [D_IN, 16]
    wrs = np.ascontiguousarray(
        W.transpose(1, 0, 2).reshape(D_IN, HF)).astype(BF16)
    in_maps = []
    for b in range(B):
        in_maps.append({
            "hT": np.ascontiguousarray(h[b].T).astype(BF16),
            "adjT": np.ascontiguousarray(adj[b].T).astype(BF16),
            "wrs": wrs,
            "w12": w12,
        })
    return in_maps


def _unscramble(arr):
    """Device layout [NPAIR*P, NIT*2*D_OUT] -> reference layout [N, H*F]."""
    a = arr.reshape(NPAIR, P, NIT, 2 * D_OUT)
    full = np.empty((N, HF), dtype=arr.dtype)
    for pr in range(NPAIR):
        if XBAR_FOLD_A:
            rows = a[pr].reshape(P * NIT, 2 * D_OUT)          # i = p*8+c
        else:
            rows = a[pr].transpose(1, 0, 2).reshape(N, 2 * D_OUT)
        full[:, 2 * pr * D_OUT:(2 * pr + 2) * D_OUT] = rows
    return full


def kernel(h, adj, W, a):
    from concourse.bass_utils import run_bass_kernel_spmd

    in_maps = _host_prep(np.asarray(h), np.asarray(adj),
                         np.asarray(W), np.asarray(a))
    nc = _build_program()
    res = run_bass_kernel_spmd(nc, in_maps, core_ids=list(range(B)))
    out = np.stack([
        _unscramble(np.asarray(res.results[b]["out"])).astype(np.float32)
        for b in range(B)])
    return out
